# revision 9
# baseline (speedup 1.0000x reference)
"""Trainium2 Bass kernel for nn_ContextualSimilarity (retrieval_knn).

kernel(z): takes FULL z [8192, 512] f32, returns FULL out [8192, 8192] f32.
8-core SPMD: 1D row-parallel + AllGather(mask) + AllToAll(column blocks).

Math (validated vs reference formulas, 0 mask flips, ~6e-8 max err):
  d2c = clamp(||z_i - z_j||^2, 0)       (compares are sqrt-free: monotone)
  M[i,j] = d2c[i,j] <= kth2_row5(i)     (reference mask == M.T)
  S = M.T M ; R = M o M.T ; T = S R ; d = diag(S) o rowsum(R)
  out = 0.5 (T/d[:,None] + T.T/d[None,:])
S,R,T entries are small ints (<256) -> bf16 matmuls exact; distances in fp32.
"""

import numpy as np

N = 8192
D = 512
P = 8

_cache = {}


def _build(n=N, d=D, stop=9):
    import concourse.bacc as bacc
    import concourse.tile as tile
    import concourse.mybir as mybir

    f32 = mybir.dt.float32
    bf16 = mybir.dt.bfloat16
    AF = mybir.ActivationFunctionType
    OP = mybir.AluOpType
    X = mybir.AxisListType.X
    RG = [list(range(P))]

    b = n // P                 # rows per core
    rt_n = b // 128            # 128-row tiles per core
    ct_n = n // 512            # 512-col tiles over n
    kt_n = n // 128            # 128-k tiles over n
    dt_n = d // 128            # 128-k tiles over d
    n2 = min(512, b)           # phase-3 col tile over b
    nt2 = b // n2
    half = n // 2              # phase-4 strip width

    nc = bacc.Bacc(
        "TRN2",
        target_bir_lowering=False,
        debug=False,
        enable_asserts=True,
        num_devices=P,
    )

    lhsA = nc.dram_tensor("lhsA", [d, b], f32, kind="ExternalInput")
    extra_lhs = nc.dram_tensor("extra_lhs", [2, b], f32, kind="ExternalInput")
    zT = nc.dram_tensor("zT", [d, n], f32, kind="ExternalInput")
    extra_rhs = nc.dram_tensor("extra_rhs", [2, n], f32, kind="ExternalInput")
    out_d = nc.dram_tensor("out", [b, n], f32, kind="ExternalOutput")

    with tile.TileContext(nc) as tc:
        with tc.tile_pool(name="dram", bufs=1, space="DRAM") as dram:
            Mloc = dram.tile([b, n], bf16)
            Mblk = dram.tile([n, b], bf16)
            Mfull = dram.tile([n, n], bf16, addr_space="Shared")
            McolY = dram.tile([n, b], bf16)
            Rloc = dram.tile([b, n], bf16)
            Rfull = dram.tile([n, n], bf16, addr_space="Shared")
            Sc = dram.tile([b, n], bf16)
            dloc = dram.tile([b], f32)
            dfull = dram.tile([n], f32, addr_space="Shared")
            TcT = dram.tile([n, b], bf16)
            Y = dram.tile([n, b], bf16)

            with tc.tile_pool(name="stats", bufs=1) as stats:
                indeg_sb = stats.tile([128, rt_n], f32)
                rsR_sb = stats.tile([128, rt_n], f32)
                d2x_sb = stats.tile([128, rt_n], f32)   # 2 * indeg * rsR
                rd_sb = stats.tile([128, rt_n], f32)    # 1 / (2 * indeg * rsR)

                # ========== Phase 1: distances + top-5 + mask ==========
                with tc.tile_pool(name="p1", bufs=1) as p1, \
                     tc.tile_pool(name="p1zt", bufs=3) as p1zt, \
                     tc.tile_pool(name="p1neg", bufs=2) as p1neg, \
                     tc.tile_pool(name="p1m", bufs=2) as p1m, \
                     tc.tile_pool(name="p1s", bufs=2) as p1s, \
                     tc.tile_pool(name="psg", bufs=4, space="PSUM") as psg:
                    lhsA_sb = p1.tile([128, dt_n, b], f32)
                    nc.sync.dma_start(
                        lhsA_sb[:], lhsA[:, :].rearrange("(kt p) m -> p kt m", p=128)
                    )
                    extra_lhs_sb = p1.tile([2, b], f32)
                    nc.sync.dma_start(extra_lhs_sb[:], extra_lhs[:, :])
                    extra_rhs_sb = p1.tile([2, n], f32)
                    nc.sync.dma_start(extra_rhs_sb[:], extra_rhs[:, :])

                    for rt in range(rt_n):
                        rsl = slice(rt * 128, (rt + 1) * 128)
                        neg = p1neg.tile([128, n], f32, name=f"neg{rt}", tag="neg")
                        for ct in range(ct_n):
                            csl = slice(ct * 512, (ct + 1) * 512)
                            zTt = p1zt.tile([128, dt_n, 512], f32,
                                            name=f"zTt{rt}_{ct}", tag="zTt")
                            nc.sync.dma_start(
                                zTt[:], zT[:, csl].rearrange("(kt p) c -> p kt c", p=128)
                            )
                            ps = psg.tile([128, 512], f32, name=f"ps{rt}_{ct}", tag="ps")
                            # sq_i + sq_j accumulates FIRST (matches the
                            # reference's  (sq_i + sq_j) - 2G  rounding order)
                            nc.tensor.matmul(
                                ps[:], extra_lhs_sb[:, rsl], extra_rhs_sb[:, csl],
                                start=True, stop=False,
                            )
                            for kt in range(dt_n):
                                nc.tensor.matmul(
                                    ps[:], lhsA_sb[:, kt, rsl], zTt[:, kt, :],
                                    start=False, stop=(kt == dt_n - 1),
                                )
                            nc.vector.tensor_scalar(
                                out=neg[:, csl], in0=ps[:], scalar1=-1.0, scalar2=0.0,
                                op0=OP.mult, op1=OP.min,
                            )
                        m8 = p1s.tile([128, 8], f32, name=f"m8_{rt}", tag="m8")
                        nc.vector.max(m8[:], neg[:])
                        Mst = p1m.tile([128, n], bf16, name=f"M{rt}", tag="M")
                        nc.vector.tensor_scalar(
                            out=Mst[:], in0=neg[:], scalar1=m8[:, 4:5], scalar2=None,
                            op0=OP.is_ge,
                        )
                        nc.sync.dma_start(Mloc[rsl, :], Mst[:])
                        for blk in range(P):
                            nc.sync.dma_start(
                                Mblk[blk * b + rt * 128: blk * b + (rt + 1) * 128, :],
                                Mst[:, blk * b:(blk + 1) * b],
                            )

                if stop >= 2:
                    nc.gpsimd.collective_compute(
                        "AllGather", OP.bypass, replica_groups=RG,
                        ins=[Mloc[:, :]], outs=[Mfull[:, :]],
                    )
                    nc.gpsimd.collective_compute(
                        "AllToAll", OP.bypass, replica_groups=RG,
                        ins=[Mblk[:, :]], outs=[McolY[:, :]],
                    )

                # ========== Phase 2a: R rows, rowsum(R), indeg, d ==========
                if stop >= 3:
                  with tc.tile_pool(name="p2r", bufs=2) as p2r:
                    for rt in range(rt_n):
                        rsl = slice(rt * 128, (rt + 1) * 128)
                        mct = p2r.tile([128, n], bf16, name=f"mct{rt}", tag="mct")
                        nc.sync.dma_start_transpose(mct[:], McolY[:, rsl])
                        mrw = p2r.tile([128, n], bf16, name=f"mrw{rt}", tag="mrw")
                        nc.sync.dma_start(mrw[:], Mloc[rsl, :])
                        rti = p2r.tile([128, n], bf16, name=f"rti{rt}", tag="rti")
                        nc.vector.tensor_tensor(
                            out=rti[:], in0=mrw[:], in1=mct[:], op=OP.mult
                        )
                        nc.vector.tensor_reduce(
                            out=rsR_sb[:, rt:rt + 1], in_=rti[:], axis=X, op=OP.add,
                        )
                        nc.sync.dma_start(Rloc[rsl, :], rti[:])
                        nc.vector.tensor_reduce(
                            out=indeg_sb[:, rt:rt + 1], in_=mct[:], axis=X, op=OP.add,
                        )
                    nc.vector.tensor_scalar(
                        out=d2x_sb[:], in0=indeg_sb[:], scalar1=2.0, scalar2=None,
                        op0=OP.mult,
                    )
                    nc.vector.tensor_tensor(
                        out=d2x_sb[:], in0=d2x_sb[:], in1=rsR_sb[:], op=OP.mult
                    )
                    nc.vector.reciprocal(rd_sb[:], d2x_sb[:])
                    nc.sync.dma_start(
                        dloc[:].rearrange("(m p) -> p m", p=128), rd_sb[:]
                    )

                if stop >= 4:
                    nc.gpsimd.collective_compute(
                        "AllGather", OP.bypass, replica_groups=RG,
                        ins=[Rloc[:, :]], outs=[Rfull[:, :]],
                    )
                    nc.gpsimd.collective_compute(
                        "AllGather", OP.bypass, replica_groups=RG,
                        ins=[dloc[:]], outs=[dfull[:]],
                    )

                # ========== Phase 2b: S_c = Mcol.T @ Mfull ==========
                if stop >= 5:
                  with tc.tile_pool(name="p2mc", bufs=1) as p2mc, \
                     tc.tile_pool(name="p2rhs", bufs=4) as p2rhs, \
                     tc.tile_pool(name="p2dr", bufs=4) as p2dr, \
                     tc.tile_pool(name="pss", bufs=rt_n, space="PSUM") as pss:
                    Mcol_sb = p2mc.tile([128, kt_n, b], bf16)
                    for kt in range(kt_n):
                        nc.sync.dma_start(
                            Mcol_sb[:, kt, :], McolY[kt * 128:(kt + 1) * 128, :]
                        )
                    for nn in range(ct_n):
                        nsl = slice(nn * 512, (nn + 1) * 512)
                        pstiles = [
                            pss.tile([128, 512], f32, name=f"psS{nn}_{m}", tag="psS")
                            for m in range(rt_n)
                        ]
                        for kt in range(kt_n):
                            rhs_t = p2rhs.tile([128, 512], bf16,
                                               name=f"rhsS{nn}_{kt}", tag="rhsS")
                            nc.sync.dma_start(
                                rhs_t[:], Mfull[kt * 128:(kt + 1) * 128, nsl]
                            )
                            for m in range(rt_n):
                                nc.tensor.matmul(
                                    pstiles[m][:],
                                    Mcol_sb[:, kt, m * 128:(m + 1) * 128],
                                    rhs_t[:],
                                    start=(kt == 0), stop=(kt == kt_n - 1),
                                )
                        for m in range(rt_n):
                            drn = p2dr.tile([128, 512], bf16,
                                            name=f"Sdr{nn}_{m}", tag="Sdr")
                            nc.scalar.activation(drn[:], pstiles[m][:], AF.Copy)
                            nc.sync.dma_start(Sc[m * 128:(m + 1) * 128, nsl], drn[:])

                # ========== Phase 3: TcT = Rfull @ Sc.T ==========
                if stop >= 6:
                  with tc.tile_pool(name="p3sct", bufs=1) as p3sct, \
                     tc.tile_pool(name="p3lhs", bufs=2) as p3lhs, \
                     tc.tile_pool(name="p3dr", bufs=4) as p3dr, \
                     tc.tile_pool(name="pst", bufs=4, space="PSUM") as pst:
                    ScT_sb = p3sct.tile([128, kt_n, b], bf16)
                    for kt in range(kt_n):
                        nc.sync.dma_start_transpose(
                            ScT_sb[:, kt, :], Sc[:, kt * 128:(kt + 1) * 128]
                        )
                    for m in range(kt_n):
                        msl = slice(m * 128, (m + 1) * 128)
                        lst = p3lhs.tile([128, kt_n, 128], bf16,
                                         name=f"lst{m}", tag="lst")
                        nc.sync.dma_start(
                            lst[:], Rfull[:, msl].rearrange("(kt p) c -> p kt c", p=128)
                        )
                        for nn in range(nt2):
                            nsl = slice(nn * n2, (nn + 1) * n2)
                            ps3 = pst.tile([128, n2], f32, name=f"psT{m}_{nn}", tag="psT")
                            for kt in range(kt_n):
                                nc.tensor.matmul(
                                    ps3[:], lst[:, kt, :], ScT_sb[:, kt, nsl],
                                    start=(kt == 0), stop=(kt == kt_n - 1),
                                )
                            dr3 = p3dr.tile([128, n2], bf16,
                                            name=f"Tdr{m}_{nn}", tag="Tdr")
                            nc.scalar.activation(dr3[:], ps3[:], AF.Copy)
                            nc.sync.dma_start(TcT[msl, nsl], dr3[:])

                if stop >= 7:
                    nc.gpsimd.collective_compute(
                        "AllToAll", OP.bypass, replica_groups=RG,
                        ins=[TcT[:, :]], outs=[Y[:, :]],
                    )

                # ========== Phase 4: out = T/(2 d_c) + T^T/(2 d) ==========
                if stop >= 8:
                  with tc.tile_pool(name="p4db", bufs=1) as p4db, \
                     tc.tile_pool(name="p4t", bufs=2) as p4t, \
                     tc.tile_pool(name="p4o", bufs=2) as p4o, \
                     tc.tile_pool(name="ps4", bufs=4, space="PSUM") as ps4:
                    dvec = p4db.tile([1, n], f32)
                    nc.sync.dma_start(
                        dvec[:], dfull[:].rearrange("(o c) -> o c", o=1)
                    )
                    ones_sb = p4db.tile([1, 128], f32)
                    nc.vector.memset(ones_sb[:], 1.0)
                    db2 = p4db.tile([128, n], f32)
                    for nn in range(ct_n):
                        nsl = slice(nn * 512, (nn + 1) * 512)
                        psb = ps4.tile([128, 512], f32, name=f"psb{nn}", tag="psb")
                        nc.tensor.matmul(psb[:], ones_sb[:], dvec[:, nsl],
                                         start=True, stop=True)
                        nc.scalar.activation(db2[:, nsl], psb[:], AF.Copy)

                    for rt in range(rt_n):
                        rsl = slice(rt * 128, (rt + 1) * 128)
                        for h in range(2):
                            hsl = slice(h * half, (h + 1) * half)
                            tc_t = p4t.tile([128, half], bf16,
                                            name=f"tc{rt}_{h}", tag="tc")
                            nc.sync.dma_start_transpose(
                                tc_t[:], TcT[hsl, rsl]
                            )
                            tt_t = p4t.tile([128, half], bf16,
                                            name=f"tt{rt}_{h}", tag="tt")
                            bph = half // b  # blocks per half strip
                            for blk in range(bph):
                                gb = h * bph + blk
                                nc.sync.dma_start(
                                    tt_t[:, blk * b:(blk + 1) * b],
                                    Y[gb * b + rt * 128: gb * b + (rt + 1) * 128, :],
                                )
                            a_t = p4o.tile([128, half], f32,
                                           name=f"a{rt}_{h}", tag="a", bufs=1)
                            nc.vector.tensor_scalar(
                                out=a_t[:], in0=tc_t[:],
                                scalar1=rd_sb[:, rt:rt + 1], scalar2=None,
                                op0=OP.mult,
                            )
                            o_t = p4o.tile([128, half], f32,
                                           name=f"o{rt}_{h}", tag="o")
                            nc.vector.tensor_tensor(
                                out=o_t[:], in0=tt_t[:], in1=db2[:, hsl], op=OP.mult
                            )
                            nc.vector.tensor_tensor(
                                out=o_t[:], in0=o_t[:], in1=a_t[:], op=OP.add
                            )
                            nc.sync.dma_start(out_d[rsl, hsl], o_t[:])

                if stop < 8:
                    with tc.tile_pool(name="pz", bufs=1) as pz:
                        zt_ = pz.tile([128, n], f32)
                        nc.vector.memset(zt_[:], 0.0)
                        for rt in range(rt_n):
                            nc.sync.dma_start(
                                out_d[rt * 128:(rt + 1) * 128, :], zt_[:]
                            )

    nc.compile()
    return nc


def _in_maps(z, n=N, d=D):
    b = n // P
    sq = np.sum(z * z, axis=1, dtype=np.float32).astype(np.float32)
    zT = np.ascontiguousarray(z.T)
    extra_rhs = np.ascontiguousarray(
        np.stack([np.ones(n, np.float32), sq]).astype(np.float32))
    maps = []
    for c in range(P):
        rows = slice(c * b, (c + 1) * b)
        maps.append({
            "lhsA": np.ascontiguousarray(-2.0 * zT[:, rows]),
            "extra_lhs": np.ascontiguousarray(
                np.stack([sq[rows], np.ones(b, np.float32)]).astype(np.float32)),
            "zT": zT,
            "extra_rhs": extra_rhs,
        })
    return maps


def run(z, n=N, d=D, trace=False):
    from concourse.bass_utils import run_bass_kernel_spmd
    z = np.ascontiguousarray(np.asarray(z, dtype=np.float32))
    assert z.shape == (n, d), z.shape
    key = (n, d)
    if key not in _cache:
        _cache[key] = _build(n, d)
    nc = _cache[key]
    res = run_bass_kernel_spmd(
        nc, _in_maps(z, n, d), core_ids=list(range(P)), trace=trace
    )
    out = np.concatenate([res.results[c]["out"] for c in range(P)], axis=0)
    return out.astype(np.float32), res


def kernel(z: np.ndarray) -> np.ndarray:
    out, _ = run(z)
    return out
